# revision 5
# baseline (speedup 1.0000x reference)
"""Grok-1 sparse MoE block (top-2 of 8 experts) on 8 TRN2 NeuronCores.

Sharding: pure expert-parallel. Core c owns expert c: it streams w0[c], wv[c],
w1[c] once, computes y_c = gelu(hs@w0c)*(hs@wvc) @ w1c for ALL 2048 tokens,
scales by the token's top-2 combine weight for expert c, and writes the
partial sum. The host adds the 8 partials (cheap) — no on-device collective.

Router trick: every core computes router logits r = hs @ gate_w, but each
core receives gate_w with its own expert's column permuted to position 0, so
the (identical SPMD) program always extracts combine-weight column 0. Core 0
gets the identity permutation, so its r IS the router_logits output.

Matmuls run as float32r (1 cycle/row when the moving dim >= 256 — 4x faster
than plain fp32, same fp32 MACs).
"""

import os
import sys

sys.path.insert(0, "/opt/trn_rl_repo")

import numpy as np

B, S, H, I, E = 1, 2048, 1024, 4096, 8
P = 128  # partitions
ST = S // P  # 16 s-tiles
HT = H // P  # 8 h-tiles
IT = I // P  # 32 i-tiles
NGRP = 8  # I groups
GI = IT // NGRP  # 4 i-tiles per group
SC = S // 512  # 4 moving chunks of 512 tokens
HC = H // 512  # 2 output-H chunks of 512

_CACHE = {}


def _build():
    import concourse.bass as bass
    import concourse.tile as tile
    from concourse import bacc, mybir
    from concourse.masks import make_identity

    f32 = mybir.dt.float32
    f32r = mybir.dt.float32r
    AF = mybir.ActivationFunctionType
    OP = mybir.AluOpType

    nc = bacc.Bacc("TRN2", target_bir_lowering=False, debug=False)

    hs_d = nc.dram_tensor("hs", [S, H], f32, kind="ExternalInput")
    gw_d = nc.dram_tensor("gate_w", [H, E], f32, kind="ExternalInput")
    w0_d = nc.dram_tensor("w0", [H, I], f32r, kind="ExternalInput")
    wv_d = nc.dram_tensor("wv", [H, I], f32r, kind="ExternalInput")
    w1_d = nc.dram_tensor("w1", [I, H], f32r, kind="ExternalInput")
    out_d = nc.dram_tensor("out_p", [S, H], f32, kind="ExternalOutput")
    r_d = nc.dram_tensor("r_out", [S, E], f32, kind="ExternalOutput")

    w0_r = w0_d[:].rearrange("(ht p) i -> p ht i", p=P)  # [128, HT, I]
    wv_r = wv_d[:].rearrange("(ht p) i -> p ht i", p=P)

    with tile.TileContext(nc) as tc:
        with (
            tc.tile_pool(name="sb", bufs=1) as sb,
            tc.tile_pool(name="stage", bufs=2) as stage,
            tc.tile_pool(name="wpool", bufs=2) as wpool,
            tc.tile_pool(name="w1pool", bufs=4) as w1pool,
            tc.tile_pool(name="ps", bufs=2, space="PSUM") as ps,
        ):
            ident = sb.tile([P, P], f32, tag="ident")
            make_identity(nc, ident[:])

            # ---- hs load + PE transpose into hsT (8 tiles of [128(H), S]) ----
            hsT = [sb.tile([P, S], f32r, tag=f"hsT{ht}", name=f"hsT{ht}") for ht in range(HT)]
            gw = sb.tile([P, HT, E], f32, tag="gw")
            nc.sync.dma_start(
                out=gw[:], in_=gw_d[:].rearrange("(ht p) e -> p ht e", p=P)
            )
            r_sb = sb.tile([P, ST, E], f32, tag="r_sb")
            comb = sb.tile([P, ST], f32, tag="comb")
            scr = sb.tile([P, ST, 8], f32, tag="scr")  # scratch per-token scalars
            for st in range(ST):
                hstile = stage.tile([P, H], f32, tag="hs_stage")
                nc.sync.dma_start(out=hstile[:], in_=hs_d[st * P : (st + 1) * P, :])
                r_lhsT = stage.tile([P, HT, P], f32, tag="r_lhsT", bufs=1)
                for ht in range(HT):
                    tp = ps.tile([P, P], f32, tag="tp")
                    nc.tensor.transpose(
                        tp[:], hstile[:, ht * P : (ht + 1) * P], ident[:]
                    )
                    nc.scalar.copy(hsT[ht][:, st * P : (st + 1) * P], tp[:])
                    nc.vector.tensor_copy(r_lhsT[:, ht, :], tp[:])
                rp = ps.tile([P, E], f32, tag="tp")
                for ht in range(HT):
                    nc.tensor.matmul(
                        rp[:],
                        r_lhsT[:, ht, :],
                        gw[:, ht, :],
                        start=(ht == 0),
                        stop=(ht == HT - 1),
                    )
                r = r_sb[:, st, :]
                nc.vector.tensor_copy(r, rp[:])
                t1 = scr[:, st, 0:1]
                t2 = scr[:, st, 1:2]
                nt1 = scr[:, st, 2:3]
                dinv = scr[:, st, 3:4]
                e0 = scr[:, st, 4:5]
                ge = scr[:, st, 5:6]
                eqm = scr[:, st, 6:8]  # unused tail
                nc.vector.tensor_reduce(t1, r, mybir.AxisListType.X, OP.max)
                # mask out one copy of the max -> second max
                rm = stage.tile([P, E], f32, tag="rm")
                eq = stage.tile([P, E], f32, tag="eq")
                nc.vector.tensor_scalar(eq[:], r, t1, None, OP.is_equal)
                nc.vector.tensor_scalar(eq[:], eq[:], 1e30, None, OP.mult)
                nc.vector.tensor_tensor(rm[:], r, eq[:], OP.subtract)
                nc.vector.tensor_reduce(t2, rm[:], mybir.AxisListType.X, OP.max)
                # dinv = 1 / (1 + exp(t2 - t1))
                nc.vector.tensor_tensor(e0, t2, t1, OP.subtract)
                nc.scalar.activation(e0, e0, AF.Exp)
                nc.vector.tensor_scalar(e0, e0, 1.0, None, OP.add)
                nc.vector.reciprocal(dinv, e0)
                # comb0 = (r0 >= t2) * exp(r0 - t1) * dinv
                nc.vector.tensor_scalar(nt1, t1, -1.0, None, OP.mult)
                x0 = r_sb[:, st, 0:1]
                nc.vector.tensor_scalar(ge, x0, t2, None, OP.is_ge)
                nc.scalar.activation(e0, x0, AF.Exp, bias=nt1)
                nc.vector.tensor_tensor(ge, ge, e0, OP.mult)
                nc.vector.tensor_tensor(comb[:, st : st + 1], ge, dinv, OP.mult)
                nc.sync.dma_start(
                    out=r_d[st * P : (st + 1) * P, :], in_=r_sb[:, st, :]
                )

            # ---- main expert pipeline over I groups ----
            y_acc = [sb.tile([P, H], f32, tag=f"y{sm}", name=f"y{sm}") for sm in range(ST)]
            act_g = [sb.tile([P, S], f32r, tag=f"act{i4}", name=f"act{i4}") for i4 in range(GI)]
            for grp in range(NGRP):
                for i4 in range(GI):
                    i = grp * GI + i4
                    w0t = wpool.tile([P, HT, P], f32r, tag="w0t")
                    wvt = wpool.tile([P, HT, P], f32r, tag="wvt")
                    nc.sync.dma_start(out=w0t[:], in_=w0_r[:, :, i * P : (i + 1) * P])
                    nc.sync.dma_start(out=wvt[:], in_=wv_r[:, :, i * P : (i + 1) * P])
                    for sc in range(SC):
                        ss = slice(sc * 512, (sc + 1) * 512)
                        hp = ps.tile([P, 512], f32, tag="hp")
                        vp = ps.tile([P, 512], f32, tag="vp")
                        for ht in range(HT):
                            nc.tensor.matmul(
                                hp[:],
                                w0t[:, ht, :],
                                hsT[ht][:, ss],
                                start=(ht == 0),
                                stop=(ht == HT - 1),
                            )
                        for ht in range(HT):
                            nc.tensor.matmul(
                                vp[:],
                                wvt[:, ht, :],
                                hsT[ht][:, ss],
                                start=(ht == 0),
                                stop=(ht == HT - 1),
                            )
                        a = act_g[i4][:, ss]
                        gelu_fn = (
                            AF.Sigmoid
                            if os.environ.get("SIM_SAFE_GELU")
                            else AF.Gelu_apprx_tanh
                        )
                        nc.scalar.activation(a, hp[:], gelu_fn)
                        nc.vector.tensor_tensor(a, a, vp[:], OP.mult)
                    # w1 tile for this i (loaded during h/v compute)
                w1t = [w1pool.tile([P, H], f32r, tag="w1t", name=f"w1t{k}") for k in range(GI)]
                for i4 in range(GI):
                    i = grp * GI + i4
                    nc.sync.dma_start(
                        out=w1t[i4][:], in_=w1_d[i * P : (i + 1) * P, :]
                    )
                for sm in range(ST):
                    for hc in range(HC):
                        yp = ps.tile([P, 512], f32, tag="yp")
                        for i4 in range(GI):
                            nc.tensor.matmul(
                                yp[:],
                                act_g[i4][:, sm * P : (sm + 1) * P],
                                w1t[i4][:, hc * 512 : (hc + 1) * 512],
                                start=(i4 == 0),
                                stop=(i4 == GI - 1),
                            )
                        dst = y_acc[sm][:, hc * 512 : (hc + 1) * 512]
                        if grp == 0:
                            nc.vector.tensor_copy(dst, yp[:])
                        else:
                            nc.vector.tensor_tensor(dst, dst, yp[:], OP.add)

            # ---- scale by combine weight, store ----
            for sm in range(ST):
                nc.vector.tensor_scalar(
                    y_acc[sm][:], y_acc[sm][:], comb[:, sm : sm + 1], None, OP.mult
                )
                nc.sync.dma_start(
                    out=out_d[sm * P : (sm + 1) * P, :], in_=y_acc[sm][:]
                )

    nc.compile()
    return nc


def _get_nc():
    if "nc" not in _CACHE:
        _CACHE["nc"] = _build()
    return _CACHE["nc"]


def _in_maps(hidden_states, gate_w, w0, wv, w1):
    hs2d = np.ascontiguousarray(
        np.asarray(hidden_states, dtype=np.float32).reshape(S, H)
    )
    gw = np.asarray(gate_w, dtype=np.float32)
    maps = []
    for c in range(E):
        perm = [c] + [j for j in range(E) if j != c]
        maps.append(
            {
                "hs": hs2d,
                "gate_w": np.ascontiguousarray(gw[:, perm]),
                "w0": np.ascontiguousarray(np.asarray(w0[c], dtype=np.float32)),
                "wv": np.ascontiguousarray(np.asarray(wv[c], dtype=np.float32)),
                "w1": np.ascontiguousarray(np.asarray(w1[c], dtype=np.float32)),
            }
        )
    return maps


def run(hidden_states, gate_w, w0, wv, w1, trace=False, trace_cores=None):
    from concourse.bass_utils import run_bass_kernel_spmd

    nc = _get_nc()
    maps = _in_maps(hidden_states, gate_w, w0, wv, w1)
    res = run_bass_kernel_spmd(
        nc,
        maps,
        core_ids=list(range(E)),
        trace=trace,
        trace_cores=trace_cores,
    )
    partials = np.stack([res.results[c]["out_p"] for c in range(E)])
    final = partials.astype(np.float64).sum(axis=0).astype(np.float32)
    final = final.reshape(B, S, H)
    router = res.results[0]["r_out"].reshape(B, S, E).astype(np.float32)
    return (final, router), res


def kernel(hidden_states, gate_w, w0, wv, w1):
    out, _ = run(hidden_states, gate_w, w0, wv, w1)
    return out


# revision 6
# speedup vs baseline: 1.0563x; 1.0563x over previous
"""Grok-1 sparse MoE block (top-2 of 8 experts) on 8 TRN2 NeuronCores.

Sharding: pure expert-parallel. Core c owns expert c: it streams w0[c], wv[c],
w1[c] once, computes y_c = gelu(hs@w0c)*(hs@wvc) @ w1c for ALL 2048 tokens,
scales by the token's top-2 combine weight for expert c, and writes the
partial sum. The host adds the 8 partials (cheap) — no on-device collective.

Router trick: every core computes router logits r = hs @ gate_w, but each
core receives gate_w with its own expert's column permuted to position 0, so
the (identical SPMD) program always extracts combine-weight column 0. Core 0
gets the identity permutation, so its r IS the router_logits output.

Matmuls run as float32r (1 cycle/row when the moving dim >= 256 — 4x faster
than plain fp32, same fp32 MACs).
"""

import os
import sys

sys.path.insert(0, "/opt/trn_rl_repo")

import numpy as np

B, S, H, I, E = 1, 2048, 1024, 4096, 8
P = 128  # partitions
ST = S // P  # 16 s-tiles
HT = H // P  # 8 h-tiles
IT = I // P  # 32 i-tiles
NGRP = 8  # I groups
GI = IT // NGRP  # 4 i-tiles per group
SC = S // 512  # 4 moving chunks of 512 tokens
HC = H // 512  # 2 output-H chunks of 512

_CACHE = {}


def _build():
    import concourse.bass as bass
    import concourse.tile as tile
    from concourse import bacc, mybir

    f32 = mybir.dt.float32
    f32r = mybir.dt.float32r
    AF = mybir.ActivationFunctionType
    OP = mybir.AluOpType

    nc = bacc.Bacc("TRN2", target_bir_lowering=False, debug=False)

    hsT_d = nc.dram_tensor("hsT", [H, S], f32r, kind="ExternalInput")
    gw_d = nc.dram_tensor("gate_w", [H, E], f32, kind="ExternalInput")
    w0_d = nc.dram_tensor("w0", [H, I], f32r, kind="ExternalInput")
    wv_d = nc.dram_tensor("wv", [H, I], f32r, kind="ExternalInput")
    w1_d = nc.dram_tensor("w1", [I, H], f32r, kind="ExternalInput")
    out_d = nc.dram_tensor("out_p", [S, H], f32, kind="ExternalOutput")
    r_d = nc.dram_tensor("r_out", [S, E], f32, kind="ExternalOutput")

    w0_r = w0_d[:].rearrange("(ht p) i -> p ht i", p=P)  # [128, HT, I]
    wv_r = wv_d[:].rearrange("(ht p) i -> p ht i", p=P)

    with tile.TileContext(nc) as tc:
        with (
            tc.tile_pool(name="sb", bufs=1) as sb,
            tc.tile_pool(name="stage", bufs=2) as stage,
            tc.tile_pool(name="wpool", bufs=2) as wpool,
            tc.tile_pool(name="w1pool", bufs=6) as w1pool,
            tc.tile_pool(name="ps", bufs=2, space="PSUM") as ps,
        ):
            # ---- hsT loaded directly (host pre-transposed), f32r chunks ----
            hsT = [sb.tile([P, S], f32r, tag=f"hsT{ht}", name=f"hsT{ht}") for ht in range(HT)]
            for ht in range(HT):
                for sc in range(SC):
                    ss = slice(sc * 512, (sc + 1) * 512)
                    nc.sync.dma_start(
                        out=hsT[ht][:, ss], in_=hsT_d[ht * P : (ht + 1) * P, ss]
                    )
            gw = sb.tile([P, HT, E], f32, tag="gw")
            nc.sync.dma_start(
                out=gw[:], in_=gw_d[:].rearrange("(ht p) e -> p ht e", p=P)
            )
            r_sb = sb.tile([P, ST, E], f32, tag="r_sb")
            comb = sb.tile([P, ST], f32, tag="comb")
            scr = sb.tile([P, ST, 8], f32, tag="scr")  # scratch per-token scalars
            for st in range(ST):
                rp = ps.tile([P, E], f32, tag="tp")
                for ht in range(HT):
                    nc.tensor.matmul(
                        rp[:],
                        hsT[ht][:, st * P : (st + 1) * P].bitcast(f32),
                        gw[:, ht, :],
                        start=(ht == 0),
                        stop=(ht == HT - 1),
                    )
                r = r_sb[:, st, :]
                nc.vector.tensor_copy(r, rp[:])
                t1 = scr[:, st, 0:1]
                t2 = scr[:, st, 1:2]
                nt1 = scr[:, st, 2:3]
                dinv = scr[:, st, 3:4]
                e0 = scr[:, st, 4:5]
                ge = scr[:, st, 5:6]
                eqm = scr[:, st, 6:8]  # unused tail
                nc.vector.tensor_reduce(t1, r, mybir.AxisListType.X, OP.max)
                # mask out one copy of the max -> second max
                rm = stage.tile([P, E], f32, tag="rm")
                eq = stage.tile([P, E], f32, tag="eq")
                nc.vector.tensor_scalar(eq[:], r, t1, None, OP.is_equal)
                nc.vector.tensor_scalar(eq[:], eq[:], 1e30, None, OP.mult)
                nc.vector.tensor_tensor(rm[:], r, eq[:], OP.subtract)
                nc.vector.tensor_reduce(t2, rm[:], mybir.AxisListType.X, OP.max)
                # dinv = 1 / (1 + exp(t2 - t1))
                nc.vector.tensor_tensor(e0, t2, t1, OP.subtract)
                nc.scalar.activation(e0, e0, AF.Exp)
                nc.vector.tensor_scalar(e0, e0, 1.0, None, OP.add)
                nc.vector.reciprocal(dinv, e0)
                # comb0 = (r0 >= t2) * exp(r0 - t1) * dinv
                nc.vector.tensor_scalar(nt1, t1, -1.0, None, OP.mult)
                x0 = r_sb[:, st, 0:1]
                nc.vector.tensor_scalar(ge, x0, t2, None, OP.is_ge)
                nc.scalar.activation(e0, x0, AF.Exp, bias=nt1)
                nc.vector.tensor_tensor(ge, ge, e0, OP.mult)
                nc.vector.tensor_tensor(comb[:, st : st + 1], ge, dinv, OP.mult)
                nc.sync.dma_start(
                    out=r_d[st * P : (st + 1) * P, :], in_=r_sb[:, st, :]
                )

            # ---- main expert pipeline over I groups ----
            y_acc = [sb.tile([P, H], f32, tag=f"y{sm}", name=f"y{sm}") for sm in range(ST)]
            act_g = [sb.tile([P, S], f32r, tag=f"act{i4}", name=f"act{i4}") for i4 in range(GI)]
            for grp in range(NGRP):
                for i4 in range(GI):
                    i = grp * GI + i4
                    w0t = wpool.tile([P, HT, P], f32r, tag="w0t")
                    wvt = wpool.tile([P, HT, P], f32r, tag="wvt")
                    nc.sync.dma_start(out=w0t[:], in_=w0_r[:, :, i * P : (i + 1) * P])
                    nc.sync.dma_start(out=wvt[:], in_=wv_r[:, :, i * P : (i + 1) * P])
                    for sc in range(SC):
                        ss = slice(sc * 512, (sc + 1) * 512)
                        hp = ps.tile([P, 512], f32, tag="hp")
                        vp = ps.tile([P, 512], f32, tag="vp")
                        for ht in range(HT):
                            nc.tensor.matmul(
                                hp[:],
                                w0t[:, ht, :],
                                hsT[ht][:, ss],
                                start=(ht == 0),
                                stop=(ht == HT - 1),
                            )
                        for ht in range(HT):
                            nc.tensor.matmul(
                                vp[:],
                                wvt[:, ht, :],
                                hsT[ht][:, ss],
                                start=(ht == 0),
                                stop=(ht == HT - 1),
                            )
                        a = act_g[i4][:, ss]
                        gelu_fn = (
                            AF.Sigmoid
                            if os.environ.get("SIM_SAFE_GELU")
                            else AF.Gelu_apprx_tanh
                        )
                        nc.scalar.activation(a, hp[:], gelu_fn)
                        nc.vector.tensor_tensor(a, a, vp[:], OP.mult)
                    # w1 tile for this i (loaded during h/v compute)
                w1t = [w1pool.tile([P, H], f32r, tag="w1t", name=f"w1t{k}") for k in range(GI)]
                for i4 in range(GI):
                    i = grp * GI + i4
                    nc.sync.dma_start(
                        out=w1t[i4][:], in_=w1_d[i * P : (i + 1) * P, :]
                    )
                for sm in range(ST):
                    for hc in range(HC):
                        yp = ps.tile([P, 512], f32, tag="yp")
                        for i4 in range(GI):
                            nc.tensor.matmul(
                                yp[:],
                                act_g[i4][:, sm * P : (sm + 1) * P],
                                w1t[i4][:, hc * 512 : (hc + 1) * 512],
                                start=(i4 == 0),
                                stop=(i4 == GI - 1),
                            )
                        dst = y_acc[sm][:, hc * 512 : (hc + 1) * 512]
                        if grp == 0:
                            nc.vector.tensor_copy(dst, yp[:])
                        else:
                            nc.vector.tensor_tensor(dst, dst, yp[:], OP.add)

            # ---- scale by combine weight, store ----
            for sm in range(ST):
                nc.vector.tensor_scalar(
                    y_acc[sm][:], y_acc[sm][:], comb[:, sm : sm + 1], None, OP.mult
                )
                nc.sync.dma_start(
                    out=out_d[sm * P : (sm + 1) * P, :], in_=y_acc[sm][:]
                )

    nc.compile()
    return nc


def _get_nc():
    if "nc" not in _CACHE:
        _CACHE["nc"] = _build()
    return _CACHE["nc"]


def _in_maps(hidden_states, gate_w, w0, wv, w1):
    hsT = np.ascontiguousarray(
        np.asarray(hidden_states, dtype=np.float32).reshape(S, H).T
    )
    gw = np.asarray(gate_w, dtype=np.float32)
    maps = []
    for c in range(E):
        perm = [c] + [j for j in range(E) if j != c]
        maps.append(
            {
                "hsT": hsT,
                "gate_w": np.ascontiguousarray(gw[:, perm]),
                "w0": np.ascontiguousarray(np.asarray(w0[c], dtype=np.float32)),
                "wv": np.ascontiguousarray(np.asarray(wv[c], dtype=np.float32)),
                "w1": np.ascontiguousarray(np.asarray(w1[c], dtype=np.float32)),
            }
        )
    return maps


def run(hidden_states, gate_w, w0, wv, w1, trace=False, trace_cores=None):
    from concourse.bass_utils import run_bass_kernel_spmd

    nc = _get_nc()
    maps = _in_maps(hidden_states, gate_w, w0, wv, w1)
    res = None
    for attempt in range(3):
        try:
            res = run_bass_kernel_spmd(
                nc,
                maps,
                core_ids=list(range(E)),
                trace=trace,
                trace_cores=trace_cores,
            )
            break
        except Exception:
            if attempt == 2:
                raise
    assert res is not None
    partials = np.stack([res.results[c]["out_p"] for c in range(E)])
    final = partials.astype(np.float64).sum(axis=0).astype(np.float32)
    final = final.reshape(B, S, H)
    router = res.results[0]["r_out"].reshape(B, S, E).astype(np.float32)
    return (final, router), res


def kernel(hidden_states, gate_w, w0, wv, w1):
    out, _ = run(hidden_states, gate_w, w0, wv, w1)
    return out
